# revision 24
# baseline (speedup 1.0000x reference)
"""Chamfer distance kernel for Trainium2 (8 NeuronCores).

Problem: input1 [4,8192,3], input2 [4,8192,3] f32.
  d2[b,n,m] = ||x_bn - y_bm||^2 (clamped at 0)
  out = mean_n(min_m d2) + mean_m(min_n d2)   (scalar f32)

Sharding: 8 cores = 4 batches x 2 halves of N. Each core computes its
4096x8192 block of the distance matrix once.

Distance tiles come from K=20 fp16 "double-double" matmuls: the K=5
augmentation  d2[n,m] = [x2,1,x].[1,y2,-2y]  with every entry split into
fp16 hi+lo parts, stacked as [ah;ah;al;al] x [bh;bl;bh;bl]. All four
partial products accumulate in fp32 PSUM: ~1e-5 abs accuracy (measured)
at full PE rate.

This environment pays a large fixed cost per STATIC instruction
(instruction fetch), so the kernel is written as dynamic hardware loops
(tc.For_i) with tiny bodies. Per 128-row group (dynamic loop over 32):
the m sweep runs in two 4096-wide halves through all 8 PSUM banks; ACT
evacuates the upper 2048 of each half to SBUF; DVE computes row mins
with chained min-scans (PSUM half, SBUF half) and folds the tile into
the column-min accumulator acc2. dist2 finale: second dynamic loop doing
PE-transposes of acc2 + one 3D row reduce per 512 columns.
Matmul weights cannot take dynamic offsets, so each iteration first
copies the row group's stationary slice into a fixed buffer.
Host merges the two N-half partial mins for dist2, clamps at 0, means.
"""

import os
import sys

import numpy as np

for _p in ("/opt/trn_rl_repo", "/root/.axon_site/_ro/trn_rl_repo"):
    if os.path.isdir(_p) and _p not in sys.path:
        sys.path.insert(0, _p)
        break

import concourse.bass as bass
import concourse.tile as tile
from concourse import mybir, bacc
from concourse.bass_utils import run_bass_kernel_spmd

B, N, M, D = 4, 8192, 8192, 3
NCORES = 8
HALF = N // 2
BIG = 3.0e38

_prog_cache: dict = {}


def build_program(
    n_rows: int = HALF, m_cols: int = M, repeat: int = 1
) -> bass.Bass:
    """One-core program. Inputs: aug [20, n_rows+m_cols] fp16 =
    [stat20(x) | mov20(y)]; ident [128,128] f32 identity. Outputs:
    out1 [n_rows] (min over m per n-row), out2 [m_cols] (min over this
    n-half per m)."""
    f32 = mybir.dt.float32
    f16 = mybir.dt.float16
    mn = mybir.AluOpType.min

    PH = 4096  # psum half-sweep width (all 8 banks)
    NT = n_rows // 128
    assert n_rows % 128 == 0 and m_cols % PH == 0
    NH = m_cols // PH  # m half-sweeps per row group (2 at full size)
    FT = m_cols // 512  # finale iterations (16)

    nc = bacc.Bacc()
    W = n_rows + m_cols
    aug = nc.declare_dram_parameter("aug", [20, W], f16, isOutput=False)
    out1 = nc.declare_dram_parameter("out1", [n_rows], f32, isOutput=True)
    out2 = nc.declare_dram_parameter("out2", [m_cols], f32, isOutput=True)

    with tile.TileContext(nc) as tc:
        with (
            tc.tile_pool(name="consts", bufs=1) as consts,
            tc.tile_pool(name="psump", bufs=1, space="PSUM") as psump,
        ):
            aug_t = consts.tile([20, W], f16)
            nc.gpsimd.dma_start(out=aug_t, in_=aug[:, :])
            as_t = aug_t[:, 0:n_rows]
            bm_t = aug_t[:, n_rows:W]

            R1 = consts.tile([128, NT], f32)
            acc2 = consts.tile([128, m_cols], f32)
            stat_buf = consts.tile([20, 128], f16)
            sts = [consts.tile([128, 2048], f32, name=f"st{h}") for h in range(NH)]
            scrs = [consts.tile([128, 2048], f32, name=f"scr{h}") for h in range(NH)]
            fold_buf = consts.tile([64, m_cols], f32)
            ps_all = psump.tile([128, PH], f32)

            nc.vector.memset(acc2, BIG)

            def main_body(i):
                # stationary slice for this row group -> fixed buffer
                nc.scalar.copy(out=stat_buf, in_=as_t[:, bass.ds(i * 128, 128)])
                prev_state = None
                for h in range(NH):
                    for q in range(PH // 512):
                        col = h * PH + q * 512
                        nc.tensor.matmul(
                            ps_all[:, q * 512 : (q + 1) * 512],
                            lhsT=stat_buf,
                            rhs=bm_t[:, col : col + 512],
                            start=True,
                            stop=True,
                        )
                    # ACT evacuates the upper half of the sweep to SBUF
                    nc.scalar.copy(out=sts[h], in_=ps_all[:, 2048:PH])
                    # row-min scan over (PSUM lower half, staged upper half)
                    nc.vector.tensor_tensor_scan(
                        out=scrs[h],
                        data0=ps_all[:, 0:2048],
                        data1=sts[h],
                        initial=BIG if prev_state is None else prev_state,
                        op0=mn,
                        op1=mn,
                    )
                    prev_state = scrs[h][:, 2047:2048]
                    # column-min accumulation
                    a_lo = acc2[:, h * PH : h * PH + 2048]
                    a_hi = acc2[:, h * PH + 2048 : (h + 1) * PH]
                    nc.vector.tensor_tensor(
                        out=a_lo, in0=ps_all[:, 0:2048], in1=a_lo, op=mn
                    )
                    nc.vector.tensor_tensor(out=a_hi, in0=sts[h], in1=a_hi, op=mn)
                nc.vector.tensor_copy(
                    out=R1[:, bass.ds(i, 1)], in_=scrs[NH - 1][:, 2047:2048]
                )

            if repeat == 1:
                with tc.For_i(0, NT, 1) as i:
                    main_body(i)
            else:
                with tc.For_i(0, repeat, 1) as _r:
                    with tc.For_i(0, NT, 1) as i:
                        main_body(i)

            # dist2 finale: fold partitions 128->1 via SBUF->SBUF DMA shifts
            # + elementwise mins; dist2 ends up in acc2[0, :]
            p = 64
            while p >= 1:
                nc.gpsimd.dma_start(out=fold_buf[0:p, :], in_=acc2[p : 2 * p, :])
                nc.vector.tensor_tensor(
                    out=acc2[0:p, :], in0=fold_buf[0:p, :], in1=acc2[0:p, :], op=mn
                )
                p //= 2

            nc.gpsimd.dma_start(out=out1[:].rearrange("(i p) -> p i", p=128), in_=R1)
            nc.gpsimd.dma_start(out=out2[:], in_=acc2[0:1, :])

    nc.finalize()
    return nc


def _get_program(n_rows: int, m_cols: int) -> bass.Bass:
    key = (n_rows, m_cols)
    if key not in _prog_cache:
        _prog_cache[key] = build_program(n_rows, m_cols)
    return _prog_cache[key]


def _aug(pts: np.ndarray):
    """pts [n,3] -> (stationary [5,n], moving [5,n]) augmented forms."""
    pts = np.asarray(pts, np.float32)
    sq = (pts * pts).sum(-1)
    ones = np.ones_like(sq)
    stat = np.ascontiguousarray(
        np.stack([sq, ones, pts[:, 0], pts[:, 1], pts[:, 2]]), dtype=np.float32
    )
    movg = np.ascontiguousarray(
        np.stack([ones, sq, -2.0 * pts[:, 0], -2.0 * pts[:, 1], -2.0 * pts[:, 2]]),
        dtype=np.float32,
    )
    return stat, movg


def _split16(a: np.ndarray):
    hi = a.astype(np.float16)
    lo = (a.astype(np.float64) - hi.astype(np.float64)).astype(np.float16)
    return hi, lo


def pack_aug(x: np.ndarray, y: np.ndarray) -> np.ndarray:
    """fp16 double-double packing: [20, n+m] = [stat20(x) | mov20(y)]."""
    a_s, _ = _aug(x)
    _, b_m = _aug(y)
    ah, al = _split16(a_s)
    bh, bl = _split16(b_m)
    stat20 = np.concatenate([ah, ah, al, al], axis=0)  # [20, n]
    mov20 = np.concatenate([bh, bl, bh, bl], axis=0)  # [20, m]
    return np.ascontiguousarray(
        np.concatenate([stat20, mov20], axis=1), dtype=np.float16
    )


def make_in_maps(input1: np.ndarray, input2: np.ndarray):
    in_maps = []
    for c in range(NCORES):
        b, h = divmod(c, 2)
        x = input1[b, h * HALF : (h + 1) * HALF]
        y = input2[b]
        in_maps.append({"aug": pack_aug(x, y)})
    return in_maps


def combine(results) -> np.ndarray:
    d1 = np.zeros((B, N), np.float32)
    d2 = np.full((B, M), np.float32(BIG), np.float32)
    for c in range(NCORES):
        b, h = divmod(c, 2)
        d1[b, h * HALF : (h + 1) * HALF] = results[c]["out1"]
        d2[b] = np.minimum(d2[b], results[c]["out2"])
    d1 = np.maximum(d1, 0.0)
    d2 = np.maximum(d2, 0.0)
    val = d1.mean(dtype=np.float64) + d2.mean(dtype=np.float64)
    return np.asarray(val, dtype=np.float32)


def run_on_hw(input1, input2, **kwargs):
    nc = _get_program(HALF, M)
    in_maps = make_in_maps(
        np.asarray(input1, np.float32), np.asarray(input2, np.float32)
    )
    return run_bass_kernel_spmd(nc, in_maps, list(range(NCORES)), **kwargs)


def kernel(input1: np.ndarray, input2: np.ndarray) -> np.ndarray:
    res = run_on_hw(input1, input2)
    return combine(res.results)
